# revision 9
# baseline (speedup 1.0000x reference)
"""MoE gate (DeepSeek-style) on 8 TRN2 cores — v3: fp16 main matmul +
term-paired fp8 DoubleRow correction.

Math (bias == 0 per spec):
    logits = x @ w.T            [T, 256] needs ~2e-5 noise for stable topk
    x = x16 + xl   (x16 = fp16(x), xl residual ~2^-11|x|)
    w = w16 + wl
    logits = x16@w16 + (x16@wl + xl@w16) + O(1e-5)
  term1 (fp16 @ fp16, 1 col/cycle):   x16 @ w16
  corr: ONE DoubleRow matmul per k-block contracts both residual terms:
        stationary (wl', wh') interleaved on the Ko axis, moving (x1, xl')
        interleaved the same way ->  sum_k x1·wl' + xl'·wh'
        with x1 = fp8(16·x16), xl' = fp8(2^14·xl), wl' = fp8(2^14·wl),
        wh' = fp8(16·w16); both products carry 2^18 -> combined * 2^-18.

Layout: stationary = weight tiles [128k x 128e] (2 expert halves), moving =
x chunks [128k x 512 tokens]; PSUM gets [expert, token] logits. Combine +
sigmoid on DVE/ACT, then PE transpose-mode flips each [128e x 128t] block
to [t, e] for the per-token routing chain on DVE.

Sharding: tokens split 8 x 2048; weight tensors replicated.
"""

import sys

if "/opt/trn_rl_repo" not in sys.path:
    sys.path.insert(0, "/opt/trn_rl_repo")

import numpy as np
import ml_dtypes

H = 7168
E = 256
TOP_K = 8
N_GROUP = 8
EPG = E // N_GROUP          # 32
TOPK_GROUP = 4
SCALING = 2.5
T_TOTAL = 16384
N_CORES = 8
T_CORE = T_TOTAL // N_CORES  # 2048
HB = H // 128                # 56 k-blocks
KQ16 = 8                     # x16 DMA slices per chunk
HBQ16 = HB // KQ16           # 7
KQ8 = 4                      # xp DMA slices per chunk
HBQ8 = HB // KQ8             # 14
CHUNK = 512                  # tokens per moving chunk
NCH = T_CORE // CHUNK        # 4
TB_PER_CH = CHUNK // 128     # 4

# fp8 scales. CRITICAL: the PE's fp8e4 treats |v| >= 256 (exponent 1111) as
# inf/NaN — e4m3-with-infinities semantics, NOT ml_dtypes' e4m3fn. All fp8
# operand values must stay strictly below 256 (we keep them <= ~90).
SX = 16.0                    # fp8 scale on x16 / w16
SL = float(2 ** 14)          # fp8 scale on residuals xl / wl
S_COMBINE = 1.0 / (SX * SL)  # 2^-18

FP8 = ml_dtypes.float8_e4m3fn

_CACHED_NC = None
LAST_RESULTS = None


def _build_nc(repeat=1, interleave=True):
    import concourse.mybir as mybir
    from concourse import bacc
    import concourse.tile as tile

    nc = bacc.Bacc("TRN2", target_bir_lowering=False, debug=False)

    f32 = mybir.dt.float32
    f16 = mybir.dt.float16
    f8 = mybir.dt.float8e4
    pm = mybir.MatmulPerfMode.DoubleRow

    x16_d = nc.dram_tensor("x16", [H, T_CORE], f16, kind="ExternalInput")
    xp_d = nc.dram_tensor("xp", [128, HB, 2, T_CORE], f8, kind="ExternalInput")
    w16_d = nc.dram_tensor("w16", [H, E], f16, kind="ExternalInput")
    wp_d = nc.dram_tensor("wp", [128, HB, 2, E], f8, kind="ExternalInput")
    id_d = nc.dram_tensor("ident", [128, 128], f32, kind="ExternalInput")
    oidx_d = nc.dram_tensor("oidx", [T_CORE, TOP_K], mybir.dt.int32,
                            kind="ExternalOutput")
    ow_d = nc.dram_tensor("ow", [T_CORE, TOP_K], f32, kind="ExternalOutput")

    with tile.TileContext(nc) as tc:
        with (
            tc.tile_pool(name="wpool", bufs=1) as wpool,
            tc.tile_pool(name="xpool", bufs=1) as xpool,
            tc.tile_pool(name="rpool", bufs=1) as rpool,
            tc.tile_pool(name="spool", bufs=1) as spool,
            tc.tile_pool(name="tpool", bufs=3) as tpool,
            tc.tile_pool(name="ppA", bufs=2, space="PSUM") as ppA,
            tc.tile_pool(name="ppB", bufs=1, space="PSUM") as ppB,
            tc.tile_pool(name="ppT", bufs=2, space="PSUM") as ppT,
        ):
            # ---- resident weights
            w16t = wpool.tile([128, HB, E], f16, tag="w16")
            nc.sync.dma_start(
                out=w16t[:], in_=w16_d[:].rearrange("(n p) e -> p n e", p=128))
            wpt = wpool.tile([128, HB, 2, E], f8, tag="wp")
            nc.sync.dma_start(out=wpt[:], in_=wp_d[:])
            ident = wpool.tile([128, 128], f32, tag="ident")
            nc.sync.dma_start(out=ident[:], in_=id_d[:])

            # scores in [token, expert] layout, all 16 t-blocks
            scoreT = spool.tile([128, T_CORE // 128, E], f32, tag="scoreT")
            idx_stage = spool.tile([128, T_CORE // 128, TOP_K], mybir.dt.uint32,
                                   tag="sidx")
            w_stage = spool.tile([128, T_CORE // 128, TOP_K], f32, tag="sw")
            if repeat == 0:
                nc.vector.memset(idx_stage[:], 0)
                nc.vector.memset(w_stage[:], 0.0)

            def emit_chunk_mms(ch):
                tsl = slice(ch * CHUNK, (ch + 1) * CHUNK)
                # x DMAs (sliced; bufs=1 ring overlaps next chunk's DMA
                # with this chunk's tail compute)
                x16q = []
                for q in range(KQ16):
                    t = xpool.tile([128, HBQ16, CHUNK], f16, tag=f"x16_{q}")
                    rsl = slice(q * HBQ16 * 128, (q + 1) * HBQ16 * 128)
                    nc.sync.dma_start(
                        out=t[:], in_=x16_d[rsl, tsl].rearrange(
                            "(n p) t -> p n t", p=128))
                    x16q.append(t)
                xpq = []
                for q in range(KQ8):
                    t = xpool.tile([128, HBQ8, 2, CHUNK], f8, tag=f"xp_{q}")
                    nsl = slice(q * HBQ8, (q + 1) * HBQ8)
                    nc.sync.dma_start(out=t[:], in_=xp_d[:, nsl, :, tsl])
                    xpq.append(t)

                # fp16 main, eh-interleaved so x16 slices free early
                psA = []
                for eh in range(2):
                    t = ppA.tile([128, CHUNK], f32, tag=f"psA{eh}")
                    psA.append(t)
                if interleave:
                    for h in range(HB):
                        q, hq = divmod(h, HBQ16)
                        for eh in range(2):
                            esl = slice(eh * 128, (eh + 1) * 128)
                            nc.tensor.matmul(
                                psA[eh][:], w16t[:, h, esl], x16q[q][:, hq, :],
                                start=(h == 0), stop=(h == HB - 1))
                else:
                    for eh in range(2):
                        esl = slice(eh * 128, (eh + 1) * 128)
                        for h in range(HB):
                            q, hq = divmod(h, HBQ16)
                            nc.tensor.matmul(
                                psA[eh][:], w16t[:, h, esl], x16q[q][:, hq, :],
                                start=(h == 0), stop=(h == HB - 1))
                # term-paired DoubleRow correction
                psB = []
                for eh in range(2):
                    t = ppB.tile([128, CHUNK], f32, tag=f"psB{eh}")
                    psB.append(t)
                if interleave:
                    for h in range(HB):
                        q, hq = divmod(h, HBQ8)
                        for eh in range(2):
                            esl = slice(eh * 128, (eh + 1) * 128)
                            nc.tensor.matmul(
                                psB[eh][:], wpt[:, h, :, esl],
                                xpq[q][:, hq, :, :],
                                start=(h == 0), stop=(h == HB - 1),
                                perf_mode=pm)
                else:
                    for eh in range(2):
                        esl = slice(eh * 128, (eh + 1) * 128)
                        for h in range(HB):
                            q, hq = divmod(h, HBQ8)
                            nc.tensor.matmul(
                                psB[eh][:], wpt[:, h, :, esl],
                                xpq[q][:, hq, :, :],
                                start=(h == 0), stop=(h == HB - 1),
                                perf_mode=pm)
                return psA, psB

            def emit_chunk_post(ch, psA, psB):
                # combine + sigmoid (DVE/ACT), then transpose each
                # [128e x 128t] f32 block to [t, e] on the DMA engines: the
                # fp32 matrix viewed as u16 pairs (lower/upper halves of
                # each fp32) is two u16 transposes — bit-exact. The xbar
                # path needs contiguous 2-byte planes, so DVE splits the
                # halves out (strided read) and merges them back after.
                u16 = mybir.dt.uint16
                for eh in range(2):
                    tmp = rpool.tile([128, CHUNK], f32, tag=f"tmp{eh}")
                    nc.vector.tensor_scalar_mul(tmp[:], psB[eh][:], S_COMBINE)
                    nc.vector.tensor_add(tmp[:], psA[eh][:], tmp[:])
                    nc.scalar.activation(
                        tmp[:], tmp[:], mybir.ActivationFunctionType.Sigmoid)
                    tmp_u = tmp[:].bitcast(u16).rearrange(
                        "p (t j) -> p j t", j=2)        # [128e, 2, 512t]
                    half = rpool.tile([128, 2, CHUNK], u16, tag=f"half{eh}")
                    nc.vector.tensor_copy(half[:], tmp_u)
                    halfT = rpool.tile([128, TB_PER_CH, 2, 128], u16,
                                       tag=f"halfT{eh}")
                    for tb2 in range(TB_PER_CH):
                        tsl = slice(tb2 * 128, (tb2 + 1) * 128)
                        for j in range(2):
                            nc.sync.dma_start_transpose(
                                out=halfT[:, tb2, j, :],
                                in_=half[:, j, tsl])
                    sco_u = scoreT[:].bitcast(u16).rearrange(
                        "p tb (e j) -> p tb e j", j=2)  # [128t, 16tb, 256e, 2]
                    esl = slice(eh * 128, (eh + 1) * 128)
                    tbsl = slice(ch * TB_PER_CH, (ch + 1) * TB_PER_CH)
                    for j in range(2):
                        nc.vector.tensor_copy(
                            sco_u[:, tbsl, esl, j], halfT[:, :, j, :])

            def emit_routing(ch):
                for tb2 in range(TB_PER_CH):
                    tb = ch * TB_PER_CH + tb2
                    sig = scoreT[:, tb, :]
                    g8 = tpool.tile([128, N_GROUP, 8], f32, tag="g8")
                    for g in range(N_GROUP):
                        nc.vector.max(out=g8[:, g, :], in_=sig[:, g * EPG:(g + 1) * EPG])
                    gs = tpool.tile([128, N_GROUP], f32, tag="gs")
                    nc.vector.tensor_add(gs[:], g8[:, :, 0], g8[:, :, 1])
                    gtop = tpool.tile([128, 8], f32, tag="gtop")
                    nc.vector.max(out=gtop[:], in_=gs[:])
                    gmask = tpool.tile([128, N_GROUP], f32, tag="gmask")
                    nc.vector.tensor_scalar(
                        gmask[:], gs[:], gtop[:, TOPK_GROUP - 1:TOPK_GROUP], None,
                        op0=mybir.AluOpType.is_ge)
                    tmp = tpool.tile([128, E], f32, tag="tmpm")
                    nc.vector.tensor_mul(
                        tmp[:].rearrange("p (g e) -> p g e", g=N_GROUP),
                        sig.rearrange("p (g e) -> p g e", g=N_GROUP),
                        gmask[:].unsqueeze(2).to_broadcast([128, N_GROUP, EPG]))
                    v8 = tpool.tile([128, TOP_K], f32, tag="v8")
                    i8 = tpool.tile([128, TOP_K], mybir.dt.uint32, tag="i8")
                    nc.vector.max(out=v8[:], in_=tmp[:])
                    nc.vector.max_index(out=i8[:], in_max=v8[:], in_values=tmp[:])
                    den = tpool.tile([128, 1], f32, tag="den")
                    nc.vector.tensor_reduce(
                        den[:], v8[:], axis=mybir.AxisListType.X,
                        op=mybir.AluOpType.add)
                    nc.vector.tensor_scalar_add(den[:], den[:], 1e-20)
                    rec = tpool.tile([128, 1], f32, tag="rec")
                    nc.vector.reciprocal(rec[:], den[:])
                    nc.vector.tensor_scalar_mul(rec[:], rec[:], SCALING)
                    nc.vector.tensor_scalar_mul(w_stage[:, tb, :], v8[:], rec[:, 0:1])
                    nc.vector.tensor_copy(idx_stage[:, tb, :], i8[:])

            for rep in range(repeat):
                prev = None
                for ch in range(NCH):
                    psA, psB = emit_chunk_mms(ch)
                    if prev is not None:
                        emit_chunk_post(prev[0], prev[1], prev[2])
                        emit_routing(prev[0])
                    prev = (ch, psA, psB)
                emit_chunk_post(prev[0], prev[1], prev[2])
                emit_routing(prev[0])

            nc.sync.dma_start(
                out=oidx_d[:].rearrange("(tb p) k -> p tb k", p=128),
                in_=idx_stage[:].bitcast(mybir.dt.int32))
            nc.sync.dma_start(
                out=ow_d[:].rearrange("(tb p) k -> p tb k", p=128),
                in_=w_stage[:])

    nc.compile()
    return nc


def build_in_maps(hidden_states, weight):
    """Host prep: transpose, fp16 split, fp8 term-paired operands, shard."""
    x = np.asarray(hidden_states, dtype=np.float32).reshape(-1, H)
    w = np.asarray(weight, dtype=np.float32)
    assert x.shape == (T_TOTAL, H) and w.shape == (E, H)

    xT = np.ascontiguousarray(x.T)                  # [H, T] fp32
    x16 = xT.astype(np.float16)
    wT = np.ascontiguousarray(w.T)                  # [H, E] fp32
    w16 = wT.astype(np.float16)
    ident = np.eye(128, dtype=np.float32)

    xl = xT - x16.astype(np.float32)
    wl = wT - w16.astype(np.float32)
    # moving pair (x1, xl'), stationary pair (wl', wh'):
    #   sum_k x1·wl' + xl'·wh'  =  2^18 (x16@wl + xl@w16)
    x1v = (x16.astype(np.float32) * SX).astype(FP8)   # [H, T]
    xlv = (xl * SL).astype(FP8)
    wlv = (wl * SL).astype(FP8)                       # [H, E]
    whv = (w16.astype(np.float32) * SX).astype(FP8)
    xp = np.stack(
        [x1v.reshape(HB, 128, T_TOTAL), xlv.reshape(HB, 128, T_TOTAL)],
        axis=2).transpose(1, 0, 2, 3)                 # [128, HB, 2, T]
    wp = np.ascontiguousarray(np.stack(
        [wlv.reshape(HB, 128, E), whv.reshape(HB, 128, E)],
        axis=2).transpose(1, 0, 2, 3))                # [128, HB, 2, E]

    in_maps = []
    for c in range(N_CORES):
        sl = slice(c * T_CORE, (c + 1) * T_CORE)
        m = {
            "x16": np.ascontiguousarray(x16[:, sl]),
            "xp": np.ascontiguousarray(xp[:, :, :, sl]),
            "w16": w16,
            "wp": wp,
            "ident": ident,
        }
        in_maps.append(m)
    return in_maps


def kernel(hidden_states, weight, e_score_correction_bias):
    global LAST_RESULTS
    from concourse.bass_utils import run_bass_kernel_spmd

    bias = np.asarray(e_score_correction_bias, dtype=np.float32)
    assert not np.any(bias), "kernel compiled for e_score_correction_bias == 0"

    in_maps = build_in_maps(hidden_states, weight)
    global _CACHED_NC
    if _CACHED_NC is None:
        _CACHED_NC = _build_nc()
    nc = _CACHED_NC
    res = None
    for attempt in range(3):
        try:
            res = run_bass_kernel_spmd(nc, in_maps, core_ids=list(range(N_CORES)))
            break
        except Exception:
            if attempt == 2:
                raise
    LAST_RESULTS = res

    topk_idx = np.concatenate([r["oidx"] for r in res.results], axis=0)
    topk_weight = np.concatenate([r["ow"] for r in res.results], axis=0)
    return topk_idx, topk_weight


# revision 10
# speedup vs baseline: 1.2886x; 1.2886x over previous
"""MoE gate (DeepSeek-style) on 8 TRN2 cores — v3: fp16 main matmul +
term-paired fp8 DoubleRow correction.

Math (bias == 0 per spec):
    logits = x @ w.T            [T, 256] needs ~2e-5 noise for stable topk
    x = x16 + xl   (x16 = fp16(x), xl residual ~2^-11|x|)
    w = w16 + wl
    logits = x16@w16 + (x16@wl + xl@w16) + O(1e-5)
  term1 (fp16 @ fp16, 1 col/cycle):   x16 @ w16
  corr: ONE DoubleRow matmul per k-block contracts both residual terms:
        stationary (wl', wh') interleaved on the Ko axis, moving (x1, xl')
        interleaved the same way ->  sum_k x1·wl' + xl'·wh'
        with x1 = fp8(16·x16), xl' = fp8(2^14·xl), wl' = fp8(2^14·wl),
        wh' = fp8(16·w16); both products carry 2^18 -> combined * 2^-18.

Layout: stationary = weight tiles [128k x 128e] (2 expert halves), moving =
x chunks [128k x 512 tokens]; PSUM gets [expert, token] logits. Combine +
sigmoid on DVE/ACT, then PE transpose-mode flips each [128e x 128t] block
to [t, e] for the per-token routing chain on DVE.

Sharding: tokens split 8 x 2048; weight tensors replicated.
"""

import sys

if "/opt/trn_rl_repo" not in sys.path:
    sys.path.insert(0, "/opt/trn_rl_repo")

import numpy as np
import ml_dtypes

H = 7168
E = 256
TOP_K = 8
N_GROUP = 8
EPG = E // N_GROUP          # 32
TOPK_GROUP = 4
SCALING = 2.5
T_TOTAL = 16384
N_CORES = 8
T_CORE = T_TOTAL // N_CORES  # 2048
HB = H // 128                # 56 k-blocks
KQ16 = 8                     # x16 DMA slices per chunk
HBQ16 = HB // KQ16           # 7
KQ8 = 4                      # xp DMA slices per chunk
HBQ8 = HB // KQ8             # 14
CHUNK = 512                  # tokens per moving chunk
NCH = T_CORE // CHUNK        # 4
TB_PER_CH = CHUNK // 128     # 4

# fp8 scales. CRITICAL: the PE's fp8e4 treats |v| >= 256 (exponent 1111) as
# inf/NaN — e4m3-with-infinities semantics, NOT ml_dtypes' e4m3fn. All fp8
# operand values must stay strictly below 256 (we keep them <= ~90).
SX = 16.0                    # fp8 scale on x16 / w16
SL = float(2 ** 14)          # fp8 scale on residuals xl / wl
S_COMBINE = 1.0 / (SX * SL)  # 2^-18

FP8 = ml_dtypes.float8_e4m3fn

_CACHED_NC = None
LAST_RESULTS = None


def _build_nc(repeat=1, interleave=True):
    import concourse.mybir as mybir
    from concourse import bacc
    import concourse.tile as tile

    nc = bacc.Bacc("TRN2", target_bir_lowering=False, debug=False)

    f32 = mybir.dt.float32
    f16 = mybir.dt.float16
    f8 = mybir.dt.float8e4
    pm = mybir.MatmulPerfMode.DoubleRow

    x16_d = nc.dram_tensor("x16", [H, T_CORE], f16, kind="ExternalInput")
    xp_d = nc.dram_tensor("xp", [128, HB, 2, T_CORE], f8, kind="ExternalInput")
    w16_d = nc.dram_tensor("w16", [H, E], f16, kind="ExternalInput")
    wp_d = nc.dram_tensor("wp", [128, HB, 2, E], f8, kind="ExternalInput")
    id_d = nc.dram_tensor("ident", [128, 128], f32, kind="ExternalInput")
    oidx_d = nc.dram_tensor("oidx", [T_CORE, TOP_K], mybir.dt.int32,
                            kind="ExternalOutput")
    ow_d = nc.dram_tensor("ow", [T_CORE, TOP_K], f32, kind="ExternalOutput")

    with tile.TileContext(nc) as tc:
        with (
            tc.tile_pool(name="wpool", bufs=1) as wpool,
            tc.tile_pool(name="xpool", bufs=1) as xpool,
            tc.tile_pool(name="rpool", bufs=1) as rpool,
            tc.tile_pool(name="spool", bufs=1) as spool,
            tc.tile_pool(name="tpool", bufs=3) as tpool,
            tc.tile_pool(name="ppA", bufs=2, space="PSUM") as ppA,
            tc.tile_pool(name="ppB", bufs=1, space="PSUM") as ppB,
            tc.tile_pool(name="ppT", bufs=2, space="PSUM") as ppT,
        ):
            # ---- resident weights
            w16t = wpool.tile([128, HB, E], f16, tag="w16")
            nc.sync.dma_start(
                out=w16t[:], in_=w16_d[:].rearrange("(n p) e -> p n e", p=128))
            wpt = wpool.tile([128, HB, 2, E], f8, tag="wp")
            nc.sync.dma_start(out=wpt[:], in_=wp_d[:])
            ident = wpool.tile([128, 128], f32, tag="ident")
            nc.sync.dma_start(out=ident[:], in_=id_d[:])

            # scores in [token, expert] layout, all 16 t-blocks
            scoreT = spool.tile([128, T_CORE // 128, E], f32, tag="scoreT")
            idx_stage = spool.tile([128, T_CORE // 128, TOP_K], mybir.dt.uint32,
                                   tag="sidx")
            w_stage = spool.tile([128, T_CORE // 128, TOP_K], f32, tag="sw")
            if repeat == 0:
                nc.vector.memset(idx_stage[:], 0)
                nc.vector.memset(w_stage[:], 0.0)

            def emit_chunk_mms(ch):
                tsl = slice(ch * CHUNK, (ch + 1) * CHUNK)
                # x DMAs (sliced; bufs=1 ring overlaps next chunk's DMA
                # with this chunk's tail compute)
                x16q = []
                for q in range(KQ16):
                    t = xpool.tile([128, HBQ16, CHUNK], f16, tag=f"x16_{q}")
                    rsl = slice(q * HBQ16 * 128, (q + 1) * HBQ16 * 128)
                    nc.sync.dma_start(
                        out=t[:], in_=x16_d[rsl, tsl].rearrange(
                            "(n p) t -> p n t", p=128))
                    x16q.append(t)
                xpq = []
                for q in range(KQ8):
                    t = xpool.tile([128, HBQ8, 2, CHUNK], f8, tag=f"xp_{q}")
                    nsl = slice(q * HBQ8, (q + 1) * HBQ8)
                    nc.sync.dma_start(out=t[:], in_=xp_d[:, nsl, :, tsl])
                    xpq.append(t)

                # fp16 main, eh-interleaved so x16 slices free early
                psA = []
                for eh in range(2):
                    t = ppA.tile([128, CHUNK], f32, tag=f"psA{eh}")
                    psA.append(t)
                if interleave:
                    for h in range(HB):
                        q, hq = divmod(h, HBQ16)
                        for eh in range(2):
                            esl = slice(eh * 128, (eh + 1) * 128)
                            nc.tensor.matmul(
                                psA[eh][:], w16t[:, h, esl], x16q[q][:, hq, :],
                                start=(h == 0), stop=(h == HB - 1))
                else:
                    for eh in range(2):
                        esl = slice(eh * 128, (eh + 1) * 128)
                        for h in range(HB):
                            q, hq = divmod(h, HBQ16)
                            nc.tensor.matmul(
                                psA[eh][:], w16t[:, h, esl], x16q[q][:, hq, :],
                                start=(h == 0), stop=(h == HB - 1))
                # term-paired DoubleRow correction
                psB = []
                for eh in range(2):
                    t = ppB.tile([128, CHUNK], f32, tag=f"psB{eh}")
                    psB.append(t)
                if interleave:
                    for h in range(HB):
                        q, hq = divmod(h, HBQ8)
                        for eh in range(2):
                            esl = slice(eh * 128, (eh + 1) * 128)
                            nc.tensor.matmul(
                                psB[eh][:], wpt[:, h, :, esl],
                                xpq[q][:, hq, :, :],
                                start=(h == 0), stop=(h == HB - 1),
                                perf_mode=pm)
                else:
                    for eh in range(2):
                        esl = slice(eh * 128, (eh + 1) * 128)
                        for h in range(HB):
                            q, hq = divmod(h, HBQ8)
                            nc.tensor.matmul(
                                psB[eh][:], wpt[:, h, :, esl],
                                xpq[q][:, hq, :, :],
                                start=(h == 0), stop=(h == HB - 1),
                                perf_mode=pm)
                return psA, psB

            def emit_chunk_post(ch, psA, psB):
                # combine + sigmoid (DVE/ACT) then PE-transpose to [t, e]
                for eh in range(2):
                    tmp = rpool.tile([128, CHUNK], f32, tag=f"tmp{eh}")
                    nc.vector.tensor_scalar_mul(tmp[:], psB[eh][:], S_COMBINE)
                    nc.vector.tensor_add(tmp[:], psA[eh][:], tmp[:])
                    sig = rpool.tile([128, CHUNK], f32, tag=f"sig{eh}")
                    nc.scalar.activation(
                        sig[:], tmp[:], mybir.ActivationFunctionType.Sigmoid)
                    for tb2 in range(TB_PER_CH):
                        tb = ch * TB_PER_CH + tb2
                        pT = ppT.tile([128, 128], f32, tag="pT")
                        nc.tensor.transpose(
                            pT[:], sig[:, tb2 * 128:(tb2 + 1) * 128], ident[:])
                        nc.scalar.copy(
                            scoreT[:, tb, eh * 128:(eh + 1) * 128], pT[:])

            def emit_routing(ch):
                for tb2 in range(TB_PER_CH):
                    tb = ch * TB_PER_CH + tb2
                    sig = scoreT[:, tb, :]
                    g8 = tpool.tile([128, N_GROUP, 8], f32, tag="g8")
                    for g in range(N_GROUP):
                        nc.vector.max(out=g8[:, g, :], in_=sig[:, g * EPG:(g + 1) * EPG])
                    gs = tpool.tile([128, N_GROUP], f32, tag="gs")
                    nc.vector.tensor_add(gs[:], g8[:, :, 0], g8[:, :, 1])
                    gtop = tpool.tile([128, 8], f32, tag="gtop")
                    nc.vector.max(out=gtop[:], in_=gs[:])
                    gmask = tpool.tile([128, N_GROUP], f32, tag="gmask")
                    nc.vector.tensor_scalar(
                        gmask[:], gs[:], gtop[:, TOPK_GROUP - 1:TOPK_GROUP], None,
                        op0=mybir.AluOpType.is_ge)
                    tmp = tpool.tile([128, E], f32, tag="tmpm")
                    nc.vector.tensor_mul(
                        tmp[:].rearrange("p (g e) -> p g e", g=N_GROUP),
                        sig.rearrange("p (g e) -> p g e", g=N_GROUP),
                        gmask[:].unsqueeze(2).to_broadcast([128, N_GROUP, EPG]))
                    v8 = tpool.tile([128, TOP_K], f32, tag="v8")
                    i8 = tpool.tile([128, TOP_K], mybir.dt.uint32, tag="i8")
                    nc.vector.max(out=v8[:], in_=tmp[:])
                    nc.vector.max_index(out=i8[:], in_max=v8[:], in_values=tmp[:])
                    den = tpool.tile([128, 1], f32, tag="den")
                    nc.vector.tensor_reduce(
                        den[:], v8[:], axis=mybir.AxisListType.X,
                        op=mybir.AluOpType.add)
                    nc.vector.tensor_scalar_add(den[:], den[:], 1e-20)
                    rec = tpool.tile([128, 1], f32, tag="rec")
                    nc.vector.reciprocal(rec[:], den[:])
                    nc.vector.tensor_scalar_mul(rec[:], rec[:], SCALING)
                    nc.vector.tensor_scalar_mul(w_stage[:, tb, :], v8[:], rec[:, 0:1])
                    nc.vector.tensor_copy(idx_stage[:, tb, :], i8[:])

            for rep in range(repeat):
                prev = None
                for ch in range(NCH):
                    psA, psB = emit_chunk_mms(ch)
                    if prev is not None:
                        emit_chunk_post(prev[0], prev[1], prev[2])
                        emit_routing(prev[0])
                    prev = (ch, psA, psB)
                emit_chunk_post(prev[0], prev[1], prev[2])
                emit_routing(prev[0])

            nc.sync.dma_start(
                out=oidx_d[:].rearrange("(tb p) k -> p tb k", p=128),
                in_=idx_stage[:].bitcast(mybir.dt.int32))
            nc.sync.dma_start(
                out=ow_d[:].rearrange("(tb p) k -> p tb k", p=128),
                in_=w_stage[:])

    nc.compile()
    return nc


def build_in_maps(hidden_states, weight):
    """Host prep: transpose, fp16 split, fp8 term-paired operands, shard."""
    x = np.asarray(hidden_states, dtype=np.float32).reshape(-1, H)
    w = np.asarray(weight, dtype=np.float32)
    assert x.shape == (T_TOTAL, H) and w.shape == (E, H)

    xT = np.ascontiguousarray(x.T)                  # [H, T] fp32
    x16 = xT.astype(np.float16)
    wT = np.ascontiguousarray(w.T)                  # [H, E] fp32
    w16 = wT.astype(np.float16)
    ident = np.eye(128, dtype=np.float32)

    xl = xT - x16.astype(np.float32)
    wl = wT - w16.astype(np.float32)
    # moving pair (x1, xl'), stationary pair (wl', wh'):
    #   sum_k x1·wl' + xl'·wh'  =  2^18 (x16@wl + xl@w16)
    x1v = (x16.astype(np.float32) * SX).astype(FP8)   # [H, T]
    xlv = (xl * SL).astype(FP8)
    wlv = (wl * SL).astype(FP8)                       # [H, E]
    whv = (w16.astype(np.float32) * SX).astype(FP8)
    xp = np.stack(
        [x1v.reshape(HB, 128, T_TOTAL), xlv.reshape(HB, 128, T_TOTAL)],
        axis=2).transpose(1, 0, 2, 3)                 # [128, HB, 2, T]
    wp = np.ascontiguousarray(np.stack(
        [wlv.reshape(HB, 128, E), whv.reshape(HB, 128, E)],
        axis=2).transpose(1, 0, 2, 3))                # [128, HB, 2, E]

    in_maps = []
    for c in range(N_CORES):
        sl = slice(c * T_CORE, (c + 1) * T_CORE)
        m = {
            "x16": np.ascontiguousarray(x16[:, sl]),
            "xp": np.ascontiguousarray(xp[:, :, :, sl]),
            "w16": w16,
            "wp": wp,
            "ident": ident,
        }
        in_maps.append(m)
    return in_maps


def kernel(hidden_states, weight, e_score_correction_bias):
    global LAST_RESULTS
    from concourse.bass_utils import run_bass_kernel_spmd

    bias = np.asarray(e_score_correction_bias, dtype=np.float32)
    assert not np.any(bias), "kernel compiled for e_score_correction_bias == 0"

    in_maps = build_in_maps(hidden_states, weight)
    global _CACHED_NC
    if _CACHED_NC is None:
        _CACHED_NC = _build_nc()
    nc = _CACHED_NC
    res = None
    for attempt in range(3):
        try:
            res = run_bass_kernel_spmd(nc, in_maps, core_ids=list(range(N_CORES)))
            break
        except Exception:
            if attempt == 2:
                raise
    LAST_RESULTS = res

    topk_idx = np.concatenate([r["oidx"] for r in res.results], axis=0)
    topk_weight = np.concatenate([r["ow"] for r in res.results], axis=0)
    return topk_idx, topk_weight


# revision 14
# speedup vs baseline: 1.4633x; 1.1355x over previous
"""MoE gate (DeepSeek-style) on 8 TRN2 cores — v3: fp16 main matmul +
term-paired fp8 DoubleRow correction.

Math (bias == 0 per spec):
    logits = x @ w.T            [T, 256] needs ~2e-5 noise for stable topk
    x = x16 + xl   (x16 = fp16(x), xl residual ~2^-11|x|)
    w = w16 + wl
    logits = x16@w16 + (x16@wl + xl@w16) + O(1e-5)
  term1 (fp16 @ fp16, 1 col/cycle):   x16 @ w16
  corr: ONE DoubleRow matmul per k-block contracts both residual terms:
        stationary (wl', wh') interleaved on the Ko axis, moving (x1, xl')
        interleaved the same way ->  sum_k x1·wl' + xl'·wh'
        with x1 = fp8(16·x16), xl' = fp8(2^14·xl), wl' = fp8(2^14·wl),
        wh' = fp8(16·w16); both products carry 2^18 -> combined * 2^-18.

Layout: stationary = weight tiles [128k x 128e] (2 expert halves), moving =
x chunks [128k x 512 tokens]; PSUM gets [expert, token] logits. Combine +
sigmoid on DVE/ACT, then PE transpose-mode flips each [128e x 128t] block
to [t, e] for the per-token routing chain on DVE.

Sharding: tokens split 8 x 2048; weight tensors replicated.
"""

import sys

if "/opt/trn_rl_repo" not in sys.path:
    sys.path.insert(0, "/opt/trn_rl_repo")

import numpy as np
import ml_dtypes

H = 7168
E = 256
TOP_K = 8
N_GROUP = 8
EPG = E // N_GROUP          # 32
TOPK_GROUP = 4
SCALING = 2.5
T_TOTAL = 16384
N_CORES = 8
T_CORE = T_TOTAL // N_CORES  # 2048
HB = H // 128                # 56 k-blocks
KQ16 = 8                     # x16 DMA slices per chunk
HBQ16 = HB // KQ16           # 7
KQ8 = 4                      # xp DMA slices per chunk
HBQ8 = HB // KQ8             # 14
CHUNK = 512                  # tokens per moving chunk
NCH = T_CORE // CHUNK        # 4
TB_PER_CH = CHUNK // 128     # 4

# fp8 scales. CRITICAL: the PE's fp8e4 treats |v| >= 256 (exponent 1111) as
# inf/NaN — e4m3-with-infinities semantics, NOT ml_dtypes' e4m3fn. All fp8
# operand values must stay strictly below 256 (we keep them <= ~90).
SX = 16.0                    # fp8 scale on x16 / w16
SL = float(2 ** 14)          # fp8 scale on residuals xl / wl
S_COMBINE = 1.0 / (SX * SL)  # 2^-18

FP8 = ml_dtypes.float8_e4m3fn

_CACHED_NC = None
LAST_RESULTS = None


def _build_nc(repeat=1, interleave=True):
    import concourse.mybir as mybir
    from concourse import bacc
    import concourse.tile as tile

    nc = bacc.Bacc("TRN2", target_bir_lowering=False, debug=False)

    f32 = mybir.dt.float32
    f16 = mybir.dt.float16
    f8 = mybir.dt.float8e4
    pm = mybir.MatmulPerfMode.DoubleRow

    x16_d = nc.dram_tensor("x16", [H, T_CORE], f16, kind="ExternalInput")
    xp_d = nc.dram_tensor("xp", [128, HB, 2, T_CORE], f8, kind="ExternalInput")
    w16_d = nc.dram_tensor("w16", [H, E], f16, kind="ExternalInput")
    wp_d = nc.dram_tensor("wp", [128, HB, 2, E], f8, kind="ExternalInput")
    id_d = nc.dram_tensor("ident", [128, 128], f32, kind="ExternalInput")
    oidx_d = nc.dram_tensor("oidx", [T_CORE, TOP_K], mybir.dt.int32,
                            kind="ExternalOutput")
    ow_d = nc.dram_tensor("ow", [T_CORE, TOP_K], f32, kind="ExternalOutput")

    with tile.TileContext(nc) as tc:
        with (
            tc.tile_pool(name="wpool", bufs=1) as wpool,
            tc.tile_pool(name="xpool", bufs=1) as xpool,
            tc.tile_pool(name="rpool", bufs=1) as rpool,
            tc.tile_pool(name="spool", bufs=1) as spool,
            tc.tile_pool(name="tpool", bufs=3) as tpool,
            tc.tile_pool(name="ppA", bufs=3, space="PSUM") as ppA,
            tc.tile_pool(name="ppT", bufs=2, space="PSUM") as ppT,
        ):
            # ---- resident weights
            w16t = wpool.tile([128, HB, E], f16, tag="w16")
            nc.sync.dma_start(
                out=w16t[:], in_=w16_d[:].rearrange("(n p) e -> p n e", p=128))
            wpt = wpool.tile([128, HB, 2, E], f8, tag="wp")
            nc.sync.dma_start(out=wpt[:], in_=wp_d[:])
            ident = wpool.tile([128, 128], f32, tag="ident")
            nc.sync.dma_start(out=ident[:], in_=id_d[:])

            # scores in [token, expert] layout, all 16 t-blocks
            scoreT = spool.tile([128, T_CORE // 128, E], f32, tag="scoreT")
            idx_stage = spool.tile([128, T_CORE // 128, TOP_K], mybir.dt.uint32,
                                   tag="sidx")
            w_stage = spool.tile([128, T_CORE // 128, TOP_K], f32, tag="sw")
            if repeat == 0:
                nc.vector.memset(idx_stage[:], 0)
                nc.vector.memset(w_stage[:], 0.0)

            def emit_chunk_mms(ch):
                tsl = slice(ch * CHUNK, (ch + 1) * CHUNK)
                # x DMAs (sliced; bufs=1 ring overlaps next chunk's DMA
                # with this chunk's tail compute)
                x16q = []
                for q in range(KQ16):
                    t = xpool.tile([128, HBQ16, CHUNK], f16, tag=f"x16_{q}")
                    rsl = slice(q * HBQ16 * 128, (q + 1) * HBQ16 * 128)
                    nc.sync.dma_start(
                        out=t[:], in_=x16_d[rsl, tsl].rearrange(
                            "(n p) t -> p n t", p=128))
                    x16q.append(t)
                xpq = []
                for q in range(KQ8):
                    t = xpool.tile([128, HBQ8, 2, CHUNK], f8, tag=f"xp_{q}")
                    nsl = slice(q * HBQ8, (q + 1) * HBQ8)
                    nc.sync.dma_start(out=t[:], in_=xp_d[:, nsl, :, tsl])
                    xpq.append(t)

                # One PSUM accumulation group per (chunk, eh): the fp16 main
                # operands are pre-scaled by 2^9 each so its products land at
                # the same 2^18 scale as the DoubleRow correction terms.
                # eh-interleaved so x16/xp slices free early for the next
                # chunk's DMA.
                psA = []
                for eh in range(2):
                    t = ppA.tile([128, CHUNK], f32, tag=f"psA{eh}")
                    psA.append(t)
                for h in range(HB):
                    q, hq = divmod(h, HBQ16)
                    for eh in range(2):
                        esl = slice(eh * 128, (eh + 1) * 128)
                        nc.tensor.matmul(
                            psA[eh][:], w16t[:, h, esl], x16q[q][:, hq, :],
                            start=(h == 0), stop=False)
                for h in range(HB):
                    q, hq = divmod(h, HBQ8)
                    for eh in range(2):
                        esl = slice(eh * 128, (eh + 1) * 128)
                        nc.tensor.matmul(
                            psA[eh][:], wpt[:, h, :, esl],
                            xpq[q][:, hq, :, :],
                            start=False, stop=(h == HB - 1),
                            perf_mode=pm)
                return psA, psA

            def emit_chunk_post(ch, psA, psB):
                # sigmoid straight from PSUM (scale folds away the 2^18)
                # then PE-transpose to [t, e]
                for eh in range(2):
                    sig = rpool.tile([128, CHUNK], f32, tag=f"sig{eh}")
                    nc.scalar.activation(
                        sig[:], psA[eh][:],
                        mybir.ActivationFunctionType.Sigmoid,
                        scale=S_COMBINE)
                    for tb2 in range(TB_PER_CH):
                        tb = ch * TB_PER_CH + tb2
                        pT = ppT.tile([128, 128], f32, tag="pT")
                        nc.tensor.transpose(
                            pT[:], sig[:, tb2 * 128:(tb2 + 1) * 128], ident[:])
                        nc.scalar.copy(
                            scoreT[:, tb, eh * 128:(eh + 1) * 128], pT[:])

            def emit_routing(ch):
                for tb2 in range(TB_PER_CH):
                    tb = ch * TB_PER_CH + tb2
                    sig = scoreT[:, tb, :]
                    g8 = tpool.tile([128, N_GROUP, 8], f32, tag="g8")
                    for g in range(N_GROUP):
                        nc.vector.max(out=g8[:, g, :], in_=sig[:, g * EPG:(g + 1) * EPG])
                    gs = tpool.tile([128, N_GROUP], f32, tag="gs")
                    nc.vector.tensor_add(gs[:], g8[:, :, 0], g8[:, :, 1])
                    gtop = tpool.tile([128, 8], f32, tag="gtop")
                    nc.vector.max(out=gtop[:], in_=gs[:])
                    gmask = tpool.tile([128, N_GROUP], f32, tag="gmask")
                    nc.vector.tensor_scalar(
                        gmask[:], gs[:], gtop[:, TOPK_GROUP - 1:TOPK_GROUP], None,
                        op0=mybir.AluOpType.is_ge)
                    tmp = tpool.tile([128, E], f32, tag="tmpm")
                    nc.vector.tensor_mul(
                        tmp[:].rearrange("p (g e) -> p g e", g=N_GROUP),
                        sig.rearrange("p (g e) -> p g e", g=N_GROUP),
                        gmask[:].unsqueeze(2).to_broadcast([128, N_GROUP, EPG]))
                    v8 = tpool.tile([128, TOP_K], f32, tag="v8")
                    i8 = tpool.tile([128, TOP_K], mybir.dt.uint32, tag="i8")
                    nc.vector.max(out=v8[:], in_=tmp[:])
                    nc.vector.max_index(out=i8[:], in_max=v8[:], in_values=tmp[:])
                    den = tpool.tile([128, 1], f32, tag="den")
                    nc.vector.tensor_reduce(
                        den[:], v8[:], axis=mybir.AxisListType.X,
                        op=mybir.AluOpType.add)
                    nc.vector.tensor_scalar_add(den[:], den[:], 1e-20)
                    rec = tpool.tile([128, 1], f32, tag="rec")
                    nc.vector.reciprocal(rec[:], den[:])
                    nc.vector.tensor_scalar_mul(rec[:], rec[:], SCALING)
                    nc.vector.tensor_scalar_mul(w_stage[:, tb, :], v8[:], rec[:, 0:1])
                    nc.vector.tensor_copy(idx_stage[:, tb, :], i8[:])

            for rep in range(repeat):
                prev = None
                for ch in range(NCH):
                    psA, psB = emit_chunk_mms(ch)
                    if prev is not None:
                        emit_chunk_post(prev[0], prev[1], prev[2])
                        emit_routing(prev[0])
                    prev = (ch, psA, psB)
                emit_chunk_post(prev[0], prev[1], prev[2])
                emit_routing(prev[0])

            nc.sync.dma_start(
                out=oidx_d[:].rearrange("(tb p) k -> p tb k", p=128),
                in_=idx_stage[:].bitcast(mybir.dt.int32))
            nc.sync.dma_start(
                out=ow_d[:].rearrange("(tb p) k -> p tb k", p=128),
                in_=w_stage[:])

    nc.compile()
    return nc


def build_in_maps(hidden_states, weight):
    """Host prep: transpose, fp16 split, fp8 term-paired operands, shard."""
    x = np.asarray(hidden_states, dtype=np.float32).reshape(-1, H)
    w = np.asarray(weight, dtype=np.float32)
    assert x.shape == (T_TOTAL, H) and w.shape == (E, H)

    xT = np.ascontiguousarray(x.T)                  # [H, T] fp32
    x16 = xT.astype(np.float16)
    wT = np.ascontiguousarray(w.T)                  # [H, E] fp32
    w16 = wT.astype(np.float16)
    ident = np.eye(128, dtype=np.float32)

    xl = xT - x16.astype(np.float32)
    wl = wT - w16.astype(np.float32)
    # moving pair (x1, xl'), stationary pair (wl', wh'):
    #   sum_k x1·wl' + xl'·wh'  =  2^18 (x16@wl + xl@w16)
    x1v = (x16.astype(np.float32) * SX).astype(FP8)   # [H, T]
    xlv = (xl * SL).astype(FP8)
    wlv = (wl * SL).astype(FP8)                       # [H, E]
    whv = (w16.astype(np.float32) * SX).astype(FP8)
    # pre-scale the fp16 main operands by 2^9 each (exact power-of-2 in
    # fp16) so main products accumulate at the same 2^18 scale as the
    # correction, sharing one PSUM group
    x16 = (x16.astype(np.float32) * 512.0).astype(np.float16)
    w16 = (w16.astype(np.float32) * 512.0).astype(np.float16)
    xp = np.stack(
        [x1v.reshape(HB, 128, T_TOTAL), xlv.reshape(HB, 128, T_TOTAL)],
        axis=2).transpose(1, 0, 2, 3)                 # [128, HB, 2, T]
    wp = np.ascontiguousarray(np.stack(
        [wlv.reshape(HB, 128, E), whv.reshape(HB, 128, E)],
        axis=2).transpose(1, 0, 2, 3))                # [128, HB, 2, E]

    in_maps = []
    for c in range(N_CORES):
        sl = slice(c * T_CORE, (c + 1) * T_CORE)
        m = {
            "x16": np.ascontiguousarray(x16[:, sl]),
            "xp": np.ascontiguousarray(xp[:, :, :, sl]),
            "w16": w16,
            "wp": wp,
            "ident": ident,
        }
        in_maps.append(m)
    return in_maps


def kernel(hidden_states, weight, e_score_correction_bias):
    global LAST_RESULTS
    from concourse.bass_utils import run_bass_kernel_spmd

    bias = np.asarray(e_score_correction_bias, dtype=np.float32)
    assert not np.any(bias), "kernel compiled for e_score_correction_bias == 0"

    in_maps = build_in_maps(hidden_states, weight)
    global _CACHED_NC
    if _CACHED_NC is None:
        _CACHED_NC = _build_nc()
    nc = _CACHED_NC
    res = None
    for attempt in range(3):
        try:
            res = run_bass_kernel_spmd(nc, in_maps, core_ids=list(range(N_CORES)))
            break
        except Exception:
            if attempt == 2:
                raise
    LAST_RESULTS = res

    topk_idx = np.concatenate([r["oidx"] for r in res.results], axis=0)
    topk_weight = np.concatenate([r["ow"] for r in res.results], axis=0)
    return topk_idx, topk_weight
